# revision 31
# baseline (speedup 1.0000x reference)
"""Trainium2 Bass kernel for nn_Decoder (MusicVAE-style hierarchical decoder).

Strategy (8 NeuronCores, data-parallel over batch, no inter-core comms):
  - Conductor LSTM: 16 sequential levels at batch 32/core. Bias + z-input
    matmuls are folded into each level's PSUM group (they don't depend on
    the recurrent h, so they issue early); all 32 gate chunks of a level
    accumulate into one [128, 4, HK, Bc] PSUM tile so the i/f/o sigmoids
    run as ONE batched activation.
  - Decoder: 16 levels are independent -> batched to 512 rows/core,
    16 sequential note steps. Recurrent (Whh), note (Wn) and output
    projections run as fp8-e4m3 DoubleRow matmuls (2 k-chunks per
    instruction at 0.5 cycles/row). The per-level conductor contribution
    ge = emb @ dec_Wih[:, :H].T + dec_b is precomputed (fp8 DoubleRow) and
    added into each step's PSUM with a bf16 identity matmul, so the gate
    comes out of PSUM complete: i/f/o sigmoid is one batched activation.
  - c state in bf16, h/note feedback quantized to fp8 (validated: rel err
    ~1.2e-2 vs the 2e-2 gate).
"""
import os
import numpy as np
import ml_dtypes

import concourse.bacc as bacc
import concourse.tile as tile
import concourse.mybir as mybir
from concourse.bass_utils import run_bass_kernel_spmd

bf16 = ml_dtypes.bfloat16
f8e4 = ml_dtypes.float8_e4m3
F32 = mybir.dt.float32
BF = mybir.dt.bfloat16
F8 = mybir.dt.float8e4
AF = mybir.ActivationFunctionType
PM = mybir.MatmulPerfMode

NCORES = 8
B, Z, H, T = 256, 512, 1024, 512
L, NS = 16, 16
Bc = B // NCORES            # 32 batch rows per core
R = L * Bc                  # 512 decoder rows per core (levels x batch)
HK, TK, ZK = H // 128, T // 128, Z // 128   # 8, 4, 4
G = 4 * H // 128            # 32 gate chunks of 128
HP = HK // 2                # 4 h k-chunk pairs (DoubleRow)
TP = TK // 2                # 2 note k-chunk pairs
NH = R // 256               # 2 moving halves of the 512 rows
GM = (0, 1, 3, 2)           # psum slot -> weight gate block (i,f,o,g)

DEC_REPS = int(os.environ.get("KBENCH_DEC_REPS", "1"))
COND_REPS = int(os.environ.get("KBENCH_COND_REPS", "1"))

PHASE_MARKS = []


def _mark(nc, name):
    try:
        PHASE_MARKS.append((name, len(nc.all_instructions())))
    except Exception:
        pass


def _declare(nc):
    d = {}
    ei = dict(kind="ExternalInput")
    d["ident"] = nc.dram_tensor("ident", [128, 128], BF, **ei)
    d["idA"] = nc.dram_tensor("idA", [128, 2, 128], F8, **ei)
    d["idB"] = nc.dram_tensor("idB", [128, 2, 128], F8, **ei)
    d["cbB"] = nc.dram_tensor("cbB", [128, G, Bc], BF, **ei)
    d["obias"] = nc.dram_tensor("obias", [128, TK], F32, **ei)
    d["zT8"] = nc.dram_tensor("zT8", [128, ZK, R], F8, **ei)
    d["h0T8"] = nc.dram_tensor("h0T8", [128, HK, R], F8, **ei)
    d["c0T"] = nc.dram_tensor("c0T", [128, HK, R], BF, **ei)
    d["cwih8"] = nc.dram_tensor("cwih8", [128, ZK, 4 * H], F8, **ei)
    d["cwhh8"] = nc.dram_tensor("cwhh8", [128, HK, 4 * H], F8, **ei)
    d["dbB"] = nc.dram_tensor("dbB", [128, G, Bc], BF, **ei)
    d["dwe8"] = nc.dram_tensor("dwe8", [128, HK, 4 * H], F8, **ei)
    d["dwhh8"] = nc.dram_tensor("dwhh8", [128, HK, 4 * H], F8, **ei)
    d["dwn8"] = nc.dram_tensor("dwn8", [128, TK, 4 * H], F8, **ei)
    d["owt8"] = nc.dram_tensor("owt8", [128, HK, T], F8, **ei)
    d["outbuf"] = nc.dram_tensor("outbuf", [NS, TK, 128, R], BF,
                                 kind="ExternalOutput")
    return d


def _body(nc, tc, d):
    import contextlib
    mm = nc.tensor.matmul
    with contextlib.ExitStack() as ctx:
        Pp = ctx.enter_context(tc.tile_pool(name="persist", bufs=1))

        t_id = Pp.tile([128, 128], BF, tag="ident")
        nc.sync.dma_start(t_id[:], d["ident"][:])
        t_idA = Pp.tile([128, 2, 128], F8, tag="idA")
        nc.sync.dma_start(t_idA[:], d["idA"][:])
        t_idB = Pp.tile([128, 2, 128], F8, tag="idB")
        nc.sync.dma_start(t_idB[:], d["idB"][:])
        t_ob = Pp.tile([128, TK], F32, tag="obias")
        nc.sync.dma_start(t_ob[:], d["obias"][:])
        t_emb8 = Pp.tile([128, HK, R], F8, tag="emb8")
        t_h8 = [Pp.tile([128, HK, R], F8, tag=f"h8{i}", name=f"h8{i}")
                for i in (0, 1)]
        t_c = Pp.tile([128, HK, R], BF, tag="c")
        t_cc = Pp.tile([128, HK, Bc], BF, tag="cc")
        t_note8 = Pp.tile([128, TK, R], F8, tag="note8")
        t_no = Pp.tile([128, TK, R], BF, tag="no")
        t_ge8 = Pp.tile([128, G, R], F8, tag="ge8")

        # ---------------- conductor ----------------
        _mark(nc, "cond")
        Pge = ctx.enter_context(tc.tile_pool(name="gew", bufs=1))
        with tc.tile_pool(name="cond", bufs=1) as Pc, \
             tc.tile_pool(name="ctmp", bufs=2) as Pt, \
             tc.tile_pool(name="cps", bufs=2, space="PSUM") as PSc, \
             tc.tile_pool(name="geps", bufs=2, space="PSUM") as PSge:
            t_cbB = Pc.tile([128, G, Bc], BF, tag="cbB")
            nc.sync.dma_start(t_cbB[:], d["cbB"][:])
            # (cbB/dbB must precede the bulk weight DMAs: level 0 needs them)
            t_dbB = Pc.tile([128, G, Bc], BF, tag="dbB")
            nc.sync.dma_start(t_dbB[:], d["dbB"][:])
            t_zT8 = Pc.tile([128, ZK, R], F8, tag="zT8")
            nc.gpsimd.dma_start(t_zT8[:], d["zT8"][:])
            t_cwih8 = Pc.tile([128, ZK, 4 * H], F8, tag="cwih8")
            for j in range(ZK // 2):
                nc.gpsimd.dma_start(t_cwih8[:, 2 * j:2 * j + 2, :],
                                    d["cwih8"][:, 2 * j:2 * j + 2, :])
            t_cwhh8 = Pc.tile([128, HK, 4 * H], F8, tag="cwhh8")
            for j in range(HP):
                nc.sync.dma_start(t_cwhh8[:, 2 * j:2 * j + 2, :],
                                  d["cwhh8"][:, 2 * j:2 * j + 2, :])
            # dwe8 is only needed at the ge phase: bulk queue, early start
            t_dwe8 = Pge.tile([128, HK, 4 * H], F8, tag="dwe8")
            for j in range(HP):
                nc.gpsimd.dma_start(t_dwe8[:, 2 * j:2 * j + 2, :],
                                    d["dwe8"][:, 2 * j:2 * j + 2, :])

            nc.gpsimd.dma_start(t_h8[0][:], d["h0T8"][:])
            nc.gpsimd.dma_start(t_c[:], d["c0T"][:])

            pending_ge = None
            for _crep in range(COND_REPS):
              for lv in range(L):
                cs = slice(lv * Bc, (lv + 1) * Bc)
                prev = slice((lv - 1) * Bc, lv * Bc)
                ps = PSc.tile([128, 4, HK, Bc], F32, tag="cps")
                for half in range(2):
                    sl = slice(half * 2 * HK, (half + 1) * 2 * HK)
                    mm(ps[:, 2 * half:2 * half + 2, :, :], t_id[:],
                       t_cbB[:, sl, :], start=True, stop=False,
                       skip_group_check=True)
                # full z sweep first: it has no dependence on the previous
                # level, so it prefires during that level's act chain instead
                # of head-blocking behind the first Whh matmul. Pair-outer so
                # level 0 starts after the first cwih8 chunk-pair DMA lands.
                for j in range(ZK // 2):
                    for s in range(4):
                        for p in range(HK):
                            m = GM[s] * HK + p
                            ms = slice(m * 128, (m + 1) * 128)
                            mm(ps[:, s, p, :],
                               t_cwih8[:, 2 * j:2 * j + 2, ms],
                               t_zT8[:, 2 * j:2 * j + 2, cs], start=False,
                               stop=(lv == 0 and j == ZK // 2 - 1),
                               perf_mode=PM.DoubleRow,
                               skip_group_check=True)
                if lv > 0:
                    for s in range(4):
                        for p in range(HK):
                            m = GM[s] * HK + p
                            ms = slice(m * 128, (m + 1) * 128)
                            for j in range(HP):
                                mm(ps[:, s, p, :],
                                   t_cwhh8[:, 2 * j:2 * j + 2, ms],
                                   t_emb8[:, 2 * j:2 * j + 2, prev],
                                   start=False, stop=(j == HP - 1),
                                   perf_mode=PM.DoubleRow,
                                   skip_group_check=True)
                if lv > 0 and pending_ge is not None:
                    pending_ge()
                    pending_ge = None
                a3 = Pt.tile([128, 3, HK, Bc], BF, tag="a3")
                ag = Pt.tile([128, HK, Bc], BF, tag="ag")
                nc.scalar.activation(a3[:], ps[:, 0:3], AF.Sigmoid)
                nc.scalar.activation(ag[:], ps[:, 3], AF.Tanh)
                tm1 = Pt.tile([128, HK, Bc], BF, tag="tm1")
                nc.vector.tensor_mul(tm1[:], a3[:, 0], ag[:])
                if lv == 0:
                    nc.vector.tensor_copy(t_cc[:], tm1[:])
                else:
                    nc.vector.tensor_mul(t_cc[:], a3[:, 1], t_cc[:])
                    nc.vector.tensor_add(t_cc[:], tm1[:], t_cc[:])
                tcn = Pt.tile([128, HK, Bc], BF, tag="ag", name="tcn")
                nc.scalar.activation(tcn[:], t_cc[:], AF.Tanh)
                nc.vector.tensor_mul(t_emb8[:, :, cs], a3[:, 2], tcn[:])

                def _ge_block(cs=cs):
                    # per-level ge = emb @ dec_Wih[:, :H].T + dec_b (fp8 DR)
                    pge = PSge.tile([128, G, Bc], F32, tag="geps",
                                    name="pge")
                    for half in range(2):
                        sl = slice(half * (G // 2), (half + 1) * (G // 2))
                        mm(pge[:, sl, :], t_id[:], t_dbB[:, sl, :],
                           start=True, stop=False, skip_group_check=True)
                    for j in range(HP):
                        for m in range(G):
                            ms = slice(m * 128, (m + 1) * 128)
                            mm(pge[:, m, :],
                               t_dwe8[:, 2 * j:2 * j + 2, ms],
                               t_emb8[:, 2 * j:2 * j + 2, cs],
                               start=False, stop=(j == HP - 1),
                               perf_mode=PM.DoubleRow,
                               skip_group_check=True)
                    nc.vector.tensor_copy(t_ge8[:, :, cs], pge[:])

                pending_ge = _ge_block

            if pending_ge is not None:
                pending_ge()
                pending_ge = None

        # ---------------- decoder weights (bulk queue) --------------------
        _mark(nc, "wdec")
        Pw = ctx.enter_context(tc.tile_pool(name="wdec", bufs=1))
        t_dwhh8 = Pw.tile([128, HK, 4 * H], F8, tag="dwhh8")
        for j in range(HP):
            nc.gpsimd.dma_start(t_dwhh8[:, 2 * j:2 * j + 2, :],
                                d["dwhh8"][:, 2 * j:2 * j + 2, :])
        t_owt8 = Pw.tile([128, HK, T], F8, tag="owt8")
        nc.gpsimd.dma_start(t_owt8[:], d["owt8"][:])
        t_dwn8 = Pw.tile([128, TK, 4 * H], F8, tag="dwn8")
        nc.gpsimd.dma_start(t_dwn8[:], d["dwn8"][:])

        # ---------------- decoder: 16 note steps over 512 rows ------------
        with tc.tile_pool(name="dtmp", bufs=2) as Pdt, \
             tc.tile_pool(name="dpsif", bufs=2, space="PSUM") as PSif, \
             tc.tile_pool(name="dpsx", bufs=4, space="PSUM") as PSx:
            for _drep in range(DEC_REPS):
              for t in range(NS):
                _mark(nc, f"dec{t:02d}")
                hin8 = t_h8[t % 2]
                hout8 = t_h8[(t + 1) % 2]
                for p in range(HK):
                    psif = PSif.tile([128, 2, R], F32, tag="psif")
                    if p == HK - 1:
                        # p7's g/o live in the psif pool: the oproj psums
                        # below then rotate onto buffers freed by p5/p6's
                        # (early) activations instead of p7's late ones
                        psgo = PSif.tile([128, 2, R], F32, tag="psif",
                                         name="psgo")
                        psx_g, psx_o = psgo[:, 0, :], psgo[:, 1, :]
                    else:
                        psx_g = PSx.tile([128, R], F32, tag="psx",
                                         name="psx_g")[:]
                        psx_o = PSx.tile([128, R], F32, tag="psx",
                                         name="psx_o")[:]
                    dsts = (psif[:, 0, :], psif[:, 1, :], psx_g, psx_o)
                    for s in range(4):
                        m = s * HK + p
                        ms = slice(m * 128, (m + 1) * 128)
                        dst = dsts[s]
                        tid = t_idA if m % 2 == 0 else t_idB
                        mlo = m if m % 2 == 0 else m - 1
                        for hh in range(NH):
                            sl = slice(hh * 256, (hh + 1) * 256)
                            mm(dst[:, sl], tid[:],
                               t_ge8[:, mlo:mlo + 2, sl],
                               start=(hh == 0), stop=False,
                               perf_mode=PM.DoubleRow,
                               skip_group_check=True)
                        for j in range(HP):
                            for hh in range(NH):
                                sl = slice(hh * 256, (hh + 1) * 256)
                                mm(dst[:, sl],
                                   t_dwhh8[:, 2 * j:2 * j + 2, ms],
                                   hin8[:, 2 * j:2 * j + 2, sl],
                                   start=False,
                                   stop=(t == 0 and j == HP - 1),
                                   perf_mode=PM.DoubleRow,
                                   skip_group_check=True)
                        if t > 0:
                            for j in range(TP):
                                for hh in range(NH):
                                    sl = slice(hh * 256, (hh + 1) * 256)
                                    mm(dst[:, sl],
                                       t_dwn8[:, 2 * j:2 * j + 2, ms],
                                       t_note8[:, 2 * j:2 * j + 2, sl],
                                       start=False, stop=(j == TP - 1),
                                       perf_mode=PM.DoubleRow,
                                       skip_group_check=True)
                    s2 = Pdt.tile([128, 2, R], BF, tag="s2")
                    tg = Pdt.tile([128, R], BF, tag="tg")
                    so = Pdt.tile([128, R], BF, tag="so")
                    nc.scalar.activation(s2[:], psif[:], AF.Sigmoid)
                    nc.scalar.activation(tg[:], psx_g, AF.Tanh)
                    nc.scalar.activation(so[:], psx_o, AF.Sigmoid)
                    tm1 = Pdt.tile([128, R], BF, tag="tm1")
                    tm2 = Pdt.tile([128, R], BF, tag="tm2")
                    nc.vector.tensor_mul(tm1[:], s2[:, 0], tg[:])
                    nc.vector.tensor_mul(tm2[:], s2[:, 1], t_c[:, p, :])
                    nc.vector.tensor_add(t_c[:, p, :], tm1[:], tm2[:])
                    if p % 2 == 1:
                        tcn = Pdt.tile([128, 2, R], BF, tag="tcn")
                        nc.scalar.activation(tcn[:], t_c[:, p - 1:p + 1, :],
                                             AF.Tanh)
                        nc.vector.tensor_mul(hout8[:, p - 1, :], so_prev[:],
                                             tcn[:, 0, :])
                        nc.vector.tensor_mul(hout8[:, p, :], so[:],
                                             tcn[:, 1, :])
                    else:
                        so_prev = so
                # output projection + sigmoid
                _mark(nc, f"oproj{t:02d}")
                nout = t_no
                for tk in range(TK):
                    ts_ = slice(tk * 128, (tk + 1) * 128)
                    po = PSx.tile([128, R], F32, tag="psx", name="po")
                    for j in range(HP):
                        for hh in range(NH):
                            sl = slice(hh * 256, (hh + 1) * 256)
                            mm(po[:, sl], t_owt8[:, 2 * j:2 * j + 2, ts_],
                               hout8[:, 2 * j:2 * j + 2, sl],
                               start=(j == 0 and hh == 0),
                               stop=(j == HP - 1),
                               perf_mode=PM.DoubleRow, skip_group_check=True)
                    nc.scalar.activation(nout[:, tk, :], po[:],
                                         AF.Sigmoid, bias=t_ob[:, tk:tk + 1])
                    nc.sync.dma_start(d["outbuf"][t, tk], nout[:, tk, :])
                    if t < NS - 1:
                        nc.vector.tensor_copy(t_note8[:, tk, :],
                                              nout[:, tk, :])


_CACHE = {}


def _build():
    if "nc" not in _CACHE:
        nc = bacc.Bacc("TRN2", target_bir_lowering=False, debug=False,
                       num_devices=NCORES)
        d = _declare(nc)
        PHASE_MARKS.clear()
        with tile.TileContext(nc) as tc:
            _body(nc, tc, d)
        nc.compile()
        _CACHE["nc"] = nc
    return _CACHE["nc"]


def _feat_major(W, dt):
    """[J, K] -> [128, K/128, J] (stationary lhsT chunk layout)."""
    J, K = W.shape
    return np.ascontiguousarray(
        W.reshape(J, K // 128, 128).transpose(2, 1, 0)).astype(dt)


def _pack_inputs(inputs):
    z = np.asarray(inputs["z"], np.float32)
    dec_h0 = np.asarray(inputs["dec_h0"], np.float32)
    dec_c0 = np.asarray(inputs["dec_c0"], np.float32)
    cond_b = np.asarray(inputs["cond_bih"] + inputs["cond_bhh"], np.float32)
    dec_b = np.asarray(inputs["dec_bih"] + inputs["dec_bhh"], np.float32)
    out_b = np.asarray(inputs["out_b"], np.float32)

    shared = {
        "ident": np.eye(128, dtype=bf16),
        "idA": np.stack([np.eye(128), np.zeros((128, 128))],
                        axis=1).astype(f8e4),
        "idB": np.stack([np.zeros((128, 128)), np.eye(128)],
                        axis=1).astype(f8e4),
        "cbB": np.ascontiguousarray(np.broadcast_to(
            cond_b.reshape(4, HK, 128)[list(GM)].reshape(G, 128)
            .T[:, :, None], (128, G, Bc))).astype(bf16),
        "obias": np.ascontiguousarray(
            out_b.reshape(TK, 128).T).astype(np.float32),
        "cwih8": _feat_major(np.asarray(inputs["cond_Wih"], np.float32),
                             f8e4),
        "cwhh8": _feat_major(np.asarray(inputs["cond_Whh"], np.float32), f8e4),
        "dbB": np.ascontiguousarray(np.broadcast_to(
            dec_b.reshape(G, 128).T[:, :, None], (128, G, Bc))).astype(bf16),
        "dwe8": _feat_major(
            np.asarray(inputs["dec_Wih"][:, :H], np.float32), f8e4),
        "dwhh8": _feat_major(np.asarray(inputs["dec_Whh"], np.float32), f8e4),
        "dwn8": _feat_major(
            np.asarray(inputs["dec_Wih"][:, H:], np.float32), f8e4),
        "owt8": _feat_major(np.asarray(inputs["out_W"], np.float32), f8e4),
    }

    z_lv = z[:, np.arange(L) * L, 0, :]           # [B, L, Z]
    in_maps = []
    for c in range(NCORES):
        bs = slice(c * Bc, (c + 1) * Bc)
        zc = z_lv[bs]                              # [Bc, L, Z]
        zT = np.ascontiguousarray(
            zc.reshape(Bc, L, ZK, 128).transpose(3, 2, 1, 0).reshape(
                128, ZK, R)).astype(f8e4)
        h0 = dec_h0[:, bs, :]                      # [L, Bc, H]
        h0T = np.ascontiguousarray(
            h0.reshape(L, Bc, HK, 128).transpose(3, 2, 0, 1).reshape(
                128, HK, R))
        c0 = dec_c0[:, bs, :]
        c0T = np.ascontiguousarray(
            c0.reshape(L, Bc, HK, 128).transpose(3, 2, 0, 1).reshape(
                128, HK, R))
        m = dict(shared)
        m["zT8"] = zT
        m["h0T8"] = h0T.astype(f8e4)
        m["c0T"] = c0T.astype(bf16)
        in_maps.append(m)
    return in_maps


def _unpack_outputs(core_outs):
    notes = np.empty((B, L * NS, T), np.float32)
    for c, arr in enumerate(core_outs):
        # arr [NS, TK, 128, R] -> [Bc, L, NS, T]
        a = arr.astype(np.float32).reshape(NS, TK, 128, L, Bc).transpose(
            4, 3, 0, 1, 2)
        notes[c * Bc:(c + 1) * Bc] = a.reshape(Bc, L, NS, T).reshape(
            Bc, L * NS, T)
    return notes


def kernel(**inputs):
    nc = _build()
    in_maps = _pack_inputs(inputs)
    res = run_bass_kernel_spmd(nc, in_maps, list(range(NCORES)))
    return _unpack_outputs([r["outbuf"] for r in res.results])
